# revision 20
# baseline (speedup 1.0000x reference)
"""Trainium2 8-core Bass kernel for a single-head causal attention layer.

Reference computation (all fp32 numpy/jax):
    Q = Xq @ Wq ; K = Xk @ Wk ; V = Xv @ Wv          # [B,S,D] @ [D,D]
    S = (Q @ K^T) / sqrt(D), causal-masked, softmax
    out = S @ V                                       # [B,S,D]
with B=4, S=2048, D=1024.

Sharding (flash-style key split, host combine): 2 cores per batch element.
The 16 key tiles (128 keys each) of a sequence are partitioned between the
pair:  even core owns T0 = {0,2,4,6,9,11,13,15}, odd core owns the
complement T1 = {1,3,5,7,8,10,12,14}.  Each core projects Q for ALL 16
query blocks but K/V only for its 8 owned key tiles, then computes the
causal-partial attention numerator N = sum_t exp(S_t)·V_t and denominator
d = sum_t exp(S_t)·1 over its own tiles.  The host combines
out = (N0 + N1) / (d0 + d1).

The ownership sets are chosen so that for every query-block pair
(2j, 2j+1) the causally needed owned tiles are exactly the first j+1
entries of the core's sorted owned list, and all but the last are strictly
below the diagonal.  Hence one SPMD instruction stream works for both
parities: pair j iterates stream tiles 0..j, multiplies a per-core
[128,256] mask (ones/triangular/zeros content is input data) at s == j
only, and accumulates with start=(s==0)/stop=(s==j).  Attention work is
68 useful key-tile iterations per core (perfectly balanced), and no K/V
projection is duplicated across the pair (saves 128 of the baseline's 928
matmul-equivalents).

Per core, all matmuls in bf16 on the TensorEngine (fp32 PSUM accumulate):
  QT[d,q]  = Wq^T Xq^T   (projections produced transposed)
  KT[d,k]  = Wk^T Xk^T   (owned key columns only)
  V [k,d]  = Xv Wv       (owned tiles)
  S^T[k,q] = sum_d KT-tile^T . QT     (scores, 2 query blocks wide)
  P^T      = exp(S^T / 32) * mask
  N        = P^T.T @ V                (unnormalized output numerator)
  den      = colsum(sum_t P^T) via a per-pair ones-matmul (DVE-accumulated)
Softmax max-subtraction is skipped: logits are ~N(0,1), far from fp32 exp
overflow, and the host-side combine is in fp32.

DMAs are emitted in first-use order so the first matmul isn't queued behind
cold data (all on the sync HWDGE queue; gpsimd SWDGE DMAs hang under axon).
"""

import sys

sys.path.insert(0, "/opt/trn_rl_repo")

import numpy as np
import ml_dtypes

import concourse.bass as bass
import concourse.mybir as mybir
import concourse.tile as tile
from concourse.tile_rust import add_dep_helper
from concourse import bacc
from concourse.bass_utils import run_bass_kernel_spmd

BF16 = mybir.dt.bfloat16
F32 = mybir.dt.float32

B, S, D = 4, 2048, 1024
P = 128
KD = D // P          # 8 contraction tiles
M = D // P           # 8 output-dim tiles
NKT = S // P         # 16 key tiles per sequence
NOWN = 8             # owned key tiles per core
N_CORES = 8
SCALE = 1.0 / float(np.sqrt(D))

# key-tile ownership per parity; for every pair (2j, 2j+1) the needed owned
# tiles are exactly own[:j+1], with own[s] < 2j for s < j
T_OWN = {
    0: [0, 2, 4, 6, 9, 11, 13, 15],
    1: [1, 3, 5, 7, 8, 10, 12, 14],
}

_cache = {}


def build_nc():
    nc = bacc.Bacc(None, target_bir_lowering=False)

    xq_e = nc.declare_dram_parameter("xq_t", [D, NOWN * P], BF16,
                                     isOutput=False)
    xk_e = nc.declare_dram_parameter("xk_t", [D, NOWN * P], BF16, isOutput=False)
    xv_e = nc.declare_dram_parameter("xv_t", [D, NOWN * P], BF16, isOutput=False)
    wq_e = nc.declare_dram_parameter("wq", [D, D], BF16, isOutput=False)
    wk_e = nc.declare_dram_parameter("wk", [D, D], BF16, isOutput=False)
    wv_e = nc.declare_dram_parameter("wv", [D, D], BF16, isOutput=False)
    mask_e = nc.declare_dram_parameter("masks", [NOWN, P, 2 * P], BF16,
                                       isOutput=False)
    outn_e = nc.declare_dram_parameter("out_n", [S, D], BF16, isOutput=True)
    # Q-projection pair exchange: each core projects its 8 q-blocks, an
    # AllGather over core pairs assembles the full QT.  gathered[h] holds
    # q-blocks 8h..8h+7 on BOTH cores (even core = pair rank 0 owns blocks
    # 0-7), so the unpack DMAs below are address-uniform across cores.
    qstage_d = nc.dram_tensor("qstage", [P, M * NOWN * P], BF16)
    qgath_d = nc.dram_tensor("qgath", [2, P, M * NOWN * P], BF16)
    # denominators packed column-wise: out_d[r, i] = den of q-row i*128+r
    outd_e = nc.declare_dram_parameter("out_d", [P, NKT], F32, isOutput=True)

    with tile.TileContext(nc) as tc:
        with (
            tc.tile_pool(name="const", bufs=1) as const,
            tc.tile_pool(name="wpool", bufs=1) as wpool,
            tc.tile_pool(name="xqpool", bufs=1) as xqpool,
            tc.tile_pool(name="xstream", bufs=8) as xstream,
            tc.tile_pool(name="vstream", bufs=8) as vstream,
        ):
            # resident destination tensors (written by projection drains)
            qt = [const.tile([P, S], BF16, tag=f"qt{m}", name=f"qt{m}")
                  for m in range(M)]
            kt = [const.tile([P, NOWN * P], BF16, tag=f"kt{m}", name=f"kt{m}")
                  for m in range(M)]
            vt = [const.tile([P, D], BF16, tag=f"vt{k}", name=f"vt{k}")
                  for k in range(NOWN)]

            def load_w(dram, wname):
                tiles = []
                for kd in range(KD):
                    wt = wpool.tile([P, D], BF16, tag=f"{wname}{kd}",
                                    name=f"{wname}{kd}")
                    nc.sync.dma_start(out=wt, in_=dram[kd * P:(kd + 1) * P, :])
                    tiles.append(wt)
                return tiles

            def load_x(dram, tag, pool):
                tiles = []
                for kd in range(KD):
                    xt = pool.tile([P, NOWN * P], BF16, tag="xs", name=tag)
                    nc.sync.dma_start(out=xt, in_=dram[kd * P:(kd + 1) * P, :])
                    tiles.append(xt)
                return tiles

            ci = 0

            def drain(out_ap, psum_ap):
                # alternate PSUM->SBUF drains between DVE and ACT
                nonlocal ci
                if ci % 2 == 0:
                    nc.vector.tensor_copy(out_ap, psum_ap)
                else:
                    nc.scalar.copy(out_ap, psum_ap)
                ci += 1

            # ---- Q projection (own 8 q-blocks only) --------------------------
            # interleave wq/xq issue so the kd-loop's operand pairs arrive
            # in consumption order (single sync queue serializes DMA issue);
            # xq is loaded in 512-col chunks so the first matmul only waits
            # for wq0 + xq0[:, 0:512].  Drains land in a staging tile that is
            # DMA'd out chunk-by-chunk; the pair AllGather then assembles the
            # full QT while the K/V projections keep TensorE busy.
            qstage = const.tile([P, M * NOWN * P], BF16, tag="qstage",
                                name="qstage")
            stage_dmas = []
            _mi = lambda i: getattr(i, "ins", i)
            wq_t, xq_tiles = [], []
            for kd in range(KD):
                wt = wpool.tile([P, D], BF16, tag=f"wq{kd}", name=f"wq{kd}")
                nc.sync.dma_start(out=wt, in_=wq_e[kd * P:(kd + 1) * P, :])
                wq_t.append(wt)
                xt = xqpool.tile([P, NOWN * P], BF16, tag=f"xq{kd}",
                                 name=f"xq{kd}")
                nc.sync.dma_start(
                    out=xt[:, 0:512],
                    in_=xq_e[kd * P:(kd + 1) * P, 0:512])
                xq_tiles.append(xt)
            for kd in range(KD):
                nc.sync.dma_start(
                    out=xq_tiles[kd][:, 512:NOWN * P],
                    in_=xq_e[kd * P:(kd + 1) * P, 512:NOWN * P])
            # K/V inputs up-front: the in-order sync queue serves them while
            # TensorE runs the Q projection, so neither the qstage chunk
            # DMAs nor pool-buffer reuse can stall the K/V projections
            wk_t = load_w(wk_e, "wk")
            xk_tiles = load_x(xk_e, "xk", xstream)
            wv_t = load_w(wv_e, "wv")
            xv_tiles = load_x(xv_e, "xv", vstream)
            with tc.tile_pool(name="ps_proj", bufs=8, space="PSUM") as ps_proj:
                for qh in range(2):          # 512-wide chunks of the own half
                    cs = slice(qh * 512, (qh + 1) * 512)
                    for g in range(2):       # m groups of 4 (PSUM budget)
                        psums = [ps_proj.tile([P, 512], F32, tag="pp", name="pp")
                                 for _ in range(4)]
                        for kd in range(KD):
                            for mi in range(4):
                                m = g * 4 + mi
                                nc.tensor.matmul(
                                    psums[mi],
                                    wq_t[kd][:, m * P:(m + 1) * P],
                                    xq_tiles[kd][:, cs],
                                    start=(kd == 0), stop=(kd == KD - 1))
                        for mi in range(4):
                            m = g * 4 + mi
                            scs = slice(m * NOWN * P + qh * 512,
                                        m * NOWN * P + (qh + 1) * 512)
                            drain(qstage[:, scs], psums[mi])
                            stage_dmas.append(nc.sync.dma_start(
                                out=qstage_d[:, scs], in_=qstage[:, scs]))
                cc = nc.gpsimd.collective_compute(
                    "AllGather",
                    mybir.AluOpType.bypass,
                    replica_groups=[[0, 1], [2, 3], [4, 5], [6, 7]],
                    ins=[qstage_d[:, :]],
                    outs=[qgath_d[:, :, :]],
                )
                # explicit edges: the DRAM-range tracker must not let the
                # collective start before every staged chunk has landed
                for sd in stage_dmas:
                    add_dep_helper(_mi(cc), _mi(sd),
                                   reason="collective reads all qstage chunks")

                # ---- K projection (owned 1024 key columns) -------------------
                for kq in range(2):          # 512-wide chunks
                    cs = slice(kq * 512, (kq + 1) * 512)
                    for g in range(2):
                        psums = [ps_proj.tile([P, 512], F32, tag="pp", name="pp")
                                 for _ in range(4)]
                        for kd in range(KD):
                            for mi in range(4):
                                m = g * 4 + mi
                                nc.tensor.matmul(
                                    psums[mi],
                                    wk_t[kd][:, m * P:(m + 1) * P],
                                    xk_tiles[kd][:, cs],
                                    start=(kd == 0), stop=(kd == KD - 1))
                        for mi in range(4):
                            drain(kt[g * 4 + mi][:, cs], psums[mi])

                # ---- V projection: V[ktile] = (Xv^T tile)^T @ Wv -------------
                for g in range(4):           # 2 key tiles per PSUM group
                    psums = [ps_proj.tile([P, 512], F32, tag="pp", name="pp")
                             for _ in range(4)]
                    for kd in range(KD):
                        for ki in range(2):
                            kloc = g * 2 + ki
                            lhsT = xv_tiles[kd][:, kloc * P:(kloc + 1) * P]
                            for ch in range(2):
                                nc.tensor.matmul(
                                    psums[ki * 2 + ch],
                                    lhsT,
                                    wv_t[kd][:, ch * 512:(ch + 1) * 512],
                                    start=(kd == 0), stop=(kd == KD - 1))
                    for ki in range(2):
                        k = g * 2 + ki
                        drain(vt[k][:, 0:512], psums[ki * 2])
                        drain(vt[k][:, 512:D], psums[ki * 2 + 1])

            # masks, needed only by the attention phase
            masks = []
            for j in range(NOWN):
                mt = const.tile([P, 2 * P], BF16, tag=f"mask{j}",
                                name=f"mask{j}")
                nc.sync.dma_start(out=mt, in_=mask_e[j])
                masks.append(mt)
            # unpack the gathered QT halves (emitted last on the sync queue:
            # these block on the collective semaphore, and everything behind
            # them -- the attention output DMAs -- runs far later)
            for h in range(2):
                for m in range(M):
                    ud = nc.sync.dma_start(
                        out=qt[m][:, h * NOWN * P:(h + 1) * NOWN * P],
                        in_=qgath_d[h][:, m * NOWN * P:(m + 1) * NOWN * P])
                    add_dep_helper(_mi(ud), _mi(cc),
                                   reason="unpack waits for pair AllGather")

            # ---- attention ---------------------------------------------------
            with (
                tc.tile_pool(name="ptp", bufs=4) as ptp,
                tc.tile_pool(name="outp", bufs=3) as outp,
                tc.tile_pool(name="accp", bufs=2) as accp,
                tc.tile_pool(name="ps_s", bufs=2, space="PSUM") as ps_s,
                tc.tile_pool(name="ps_o", bufs=2, space="PSUM") as ps_o,
                tc.tile_pool(name="ps_d", bufs=2, space="PSUM") as ps_d,
            ):
                # packed denominators: column 2j+sl = pair j slot sl
                dt_ = const.tile([P, NKT], F32, tag="dt", name="dt")
                ones = const.tile([P, 1], BF16, tag="ones", name="ones")
                nc.vector.memset(ones, 1.0)
                # ascending j: early (small) pairs drain while later pairs
                # compute; the tail is a single 2-block epilogue
                for j in range(NOWN):
                    po = [ps_o.tile([P, D], F32, tag="po", name="po")
                          for _ in range(2)]
                    pd = [ps_d.tile([P, 1], F32, tag="pd", name="pd")
                          for _ in range(2)]
                    acc = accp.tile([P, 256], F32, tag="acc", name="acc")
                    qcs = slice(2 * j * P, (2 * j + 2) * P)
                    for s in range(j + 1):
                        ps = ps_s.tile([P, 256], F32, tag="ps", name="ps")
                        for m in range(M):
                            nc.tensor.matmul(
                                ps,
                                kt[m][:, s * P:(s + 1) * P],
                                qt[m][:, qcs],
                                start=(m == 0), stop=(m == M - 1))
                        pt = ptp.tile([P, 256], BF16, tag="pt", name="pt")
                        nc.scalar.activation(
                            pt, ps, mybir.ActivationFunctionType.Exp,
                            scale=SCALE)
                        if s == j:
                            nc.vector.tensor_mul(pt, pt, masks[j])
                        # denominator: accumulate exp weights on DVE (f32)
                        # instead of a per-iteration TensorE ones-matmul
                        if s == 0:
                            nc.vector.tensor_copy(acc, pt)
                        else:
                            nc.vector.tensor_add(acc, acc, pt)
                        for sl in range(2):
                            lhsT = pt[:, sl * P:(sl + 1) * P]
                            for ch in range(2):
                                nc.tensor.matmul(
                                    po[sl][:, ch * 512:(ch + 1) * 512],
                                    lhsT,
                                    vt[s][:, ch * 512:(ch + 1) * 512],
                                    start=(s == 0), stop=(s == j))
                    # per-pair column-sum of acc via two tiny matmuls
                    db = ptp.tile([P, 256], BF16, tag="db", name="db")
                    nc.vector.tensor_copy(db, acc)
                    for sl in range(2):
                        nc.tensor.matmul(
                            pd[sl], db[:, sl * P:(sl + 1) * P], ones,
                            start=True, stop=True)
                    # epilogue drains on DVE (idle between mask-muls; the next
                    # mask is a full pair-duration away) + tiny pd on ACT.
                    # For the final pair: emit out_d before the last block's
                    # numerator DMA and split that DMA across two engines.
                    last = j == NOWN - 1
                    for sl in range(2):
                        r = (2 * j + sl) * P
                        ot = outp.tile([P, D], BF16, tag="ot", name="ot")
                        nc.vector.tensor_copy(ot[:, 0:512], po[sl][:, 0:512])
                        if last and sl == 1:
                            nc.scalar.copy(ot[:, 512:D], po[sl][:, 512:D])
                        else:
                            nc.vector.tensor_copy(ot[:, 512:D],
                                                  po[sl][:, 512:D])
                        nc.scalar.copy(
                            dt_[:, 2 * j + sl:2 * j + sl + 1], pd[sl])
                        if last and sl == 1:
                            nc.sync.dma_start(out=outd_e[:, :], in_=dt_)
                            nc.sync.dma_start(
                                out=outn_e[r:r + P, 0:512], in_=ot[:, 0:512])
                            nc.sync.dma_start(
                                out=outn_e[r:r + P, 512:D], in_=ot[:, 512:D])
                        else:
                            nc.sync.dma_start(out=outn_e[r:r + P, :], in_=ot)

    nc.finalize()
    return nc


def _make_masks(parity):
    tri = np.triu(np.ones((P, P), np.float32))     # keep k <= q  ([k,q] layout)
    m = np.zeros((NOWN, P, 2 * P), np.float32)
    own = T_OWN[parity]
    for j in range(NOWN):
        p = own[j]
        for sl in range(2):
            i = 2 * j + sl
            if p < i:
                m[j, :, sl * P:(sl + 1) * P] = 1.0
            elif p == i:
                m[j, :, sl * P:(sl + 1) * P] = tri
    return m


def _prep_inputs(inputs_for_keys, inputs_for_values, inputs_for_queries,
                 W_k, W_v, W_q):
    bf = ml_dtypes.bfloat16
    wq = np.ascontiguousarray(W_q.astype(bf))
    wk = np.ascontiguousarray(W_k.astype(bf))
    wv = np.ascontiguousarray(W_v.astype(bf))
    masks = {p: _make_masks(p).astype(bf) for p in range(2)}

    in_maps = []
    for c in range(N_CORES):
        b, parity = divmod(c, 2)
        own = T_OWN[parity]
        rows = np.concatenate([np.arange(t * P, (t + 1) * P) for t in own])
        qrows = slice(parity * NOWN * P, (parity + 1) * NOWN * P)
        in_maps.append({
            "xq_t": np.ascontiguousarray(
                inputs_for_queries[b][qrows].T).astype(bf),
            "xk_t": np.ascontiguousarray(inputs_for_keys[b][rows].T).astype(bf),
            "xv_t": np.ascontiguousarray(inputs_for_values[b][rows].T).astype(bf),
            "wq": wq, "wk": wk, "wv": wv,
            "masks": masks[parity],
        })
    return in_maps


def _unshard(results):
    out = np.empty((B, S, D), np.float32)
    for b in range(B):
        n0 = np.asarray(results[2 * b]["out_n"], np.float32)
        n1 = np.asarray(results[2 * b + 1]["out_n"], np.float32)
        # out_d[r, i] = den of q-row i*128+r -> flatten to [S, 1]
        d0 = np.asarray(results[2 * b]["out_d"], np.float32)
        d1 = np.asarray(results[2 * b + 1]["out_d"], np.float32)
        d = (d0 + d1).T.reshape(S, 1)
        out[b] = (n0 + n1) / d
    return out


def kernel(inputs_for_keys, inputs_for_values, inputs_for_queries,
           W_k, W_v, W_q):
    inputs_for_keys = np.asarray(inputs_for_keys, np.float32)
    inputs_for_values = np.asarray(inputs_for_values, np.float32)
    inputs_for_queries = np.asarray(inputs_for_queries, np.float32)
    W_k = np.asarray(W_k, np.float32)
    W_v = np.asarray(W_v, np.float32)
    W_q = np.asarray(W_q, np.float32)

    if "nc" not in _cache:
        _cache["nc"] = build_nc()
    nc = _cache["nc"]

    in_maps = _prep_inputs(inputs_for_keys, inputs_for_values,
                           inputs_for_queries, W_k, W_v, W_q)
    res = run_bass_kernel_spmd(nc, in_maps, core_ids=list(range(N_CORES)))
    return _unshard(res.results)
